# revision 1
# baseline (speedup 1.0000x reference)
"""DenoiseLSTM Trainium2 kernel (8 NeuronCores, SPMD).

Strategy: the recurrent parts (bi-LSTM encoder, LSTM decoder) are replicated
on all 8 cores (they are latency-bound and do not parallelize profitably);
the dominant vocab projection [B*T,512]@[512,32000] is sharded over V
(4000 columns per core). No collectives; the host concatenates V-shards.

All recurrences run in transposed layout (h.T chunks [128, B]) so the LSTM
elementwise uses all 128 partitions and h.T is directly the next step's
matmul rhs. Input projections x@Wih.T are batched up front into DRAM and
streamed per step. Attention + FFN + vocab projection are batched after the
decoder scan (they do not feed the recurrence).
"""
import sys

sys.path.insert(0, "/opt/trn_rl_repo")

from contextlib import ExitStack

import numpy as np
import ml_dtypes

import concourse.bass as bass
import concourse.bacc as bacc
import concourse.mybir as mybir
import concourse.tile as tile
from concourse.bass_utils import run_bass_kernel_spmd
from concourse.masks import make_identity

bf16 = ml_dtypes.bfloat16
F32 = mybir.dt.float32
BF16 = mybir.dt.bfloat16
I16 = mybir.dt.int16
AF = mybir.ActivationFunctionType
ALU = mybir.AluOpType
AX = mybir.AxisListType

B = 32
D_EMB = 128
D_ENC = 256
D_DEC = 512
N_CORES = 8


class _Stop(Exception):
    pass


def build(S=128, T=128, V=32000, VS=4000, phases=6):
    """Builds + compiles the Bacc module. Returns nc."""
    NI_E = B * S          # encoder gather count
    NI_D = B * T          # decoder gather count
    assert NI_E % 128 == 0 and NI_D % 128 == 0
    assert (B * S) % 512 == 0 and (B * T) % 512 == 0
    KD = D_DEC // 128     # 4 hidden chunks (decoder)
    KE = D_ENC // 128     # 2 hidden chunks (encoder)
    MD = 4 * D_DEC // 128  # 16 gate chunks (decoder)
    ME = 4 * D_ENC // 128  # 8 gate chunks (encoder)
    NBLK_E = (B * S) // 512
    NBLK_D = (B * T) // 512
    NVC = (VS + 499) // 500  # vocab column chunks per core
    SCALE = 1.0 / float(np.sqrt(np.float32(2 * D_ENC)))

    nc = bacc.Bacc("TRN2", target_bir_lowering=False, debug=False)

    # ---- external inputs (host-prepped layouts) ----
    tokb = nc.dram_tensor("tokb", [V, D_EMB], BF16, kind="ExternalInput")
    idx_e = nc.dram_tensor("idx_e", [128, NI_E // 16], I16, kind="ExternalInput")
    idx_d = nc.dram_tensor("idx_d", [128, NI_D // 16], I16, kind="ExternalInput")
    startT = nc.dram_tensor("startT", [128, 1], BF16, kind="ExternalInput")
    # encoder init: h0 = e0 + lab*(e1-e0), per direction (fwd = cols 0:256, bwd 256:512)
    diff_e = nc.dram_tensor("diff_e", [1, 2 * D_ENC], BF16, kind="ExternalInput")
    e0T = nc.dram_tensor("e0T", [128, KD], F32, kind="ExternalInput")
    lab_i = nc.dram_tensor("lab_i", [1, B], BF16, kind="ExternalInput")
    # decoder init h_t from style_emb
    diff_s = nc.dram_tensor("diff_s", [1, D_DEC], BF16, kind="ExternalInput")
    s0T = nc.dram_tensor("s0T", [128, KD], F32, kind="ExternalInput")
    lab_d = nc.dram_tensor("lab_d", [1, B], BF16, kind="ExternalInput")
    # weights, PE-ready (lhsT tiles chained along free dim, (k, m)-major)
    wih_f = nc.dram_tensor("wih_f", [128, ME * 128], BF16, kind="ExternalInput")
    wih_b = nc.dram_tensor("wih_b", [128, ME * 128], BF16, kind="ExternalInput")
    wih_d = nc.dram_tensor("wih_d", [128, MD * 128], BF16, kind="ExternalInput")
    whh_f = nc.dram_tensor("whh_f", [128, KE * ME * 128], BF16, kind="ExternalInput")
    whh_b = nc.dram_tensor("whh_b", [128, KE * ME * 128], BF16, kind="ExternalInput")
    whh_d = nc.dram_tensor("whh_d", [128, KD * MD * 128], BF16, kind="ExternalInput")
    wtr = nc.dram_tensor("wtr", [128, KD * KD * 128], BF16, kind="ExternalInput")
    wf1 = nc.dram_tensor("wf1", [128, 8 * KD * 128], BF16, kind="ExternalInput")
    wf2 = nc.dram_tensor("wf2", [128, KD * VS], BF16, kind="ExternalInput")
    # bias columns (bih+bhh summed on host); b1h = 0.55 * b_f1
    bs_f = nc.dram_tensor("bs_f", [128, ME], F32, kind="ExternalInput")
    bs_b = nc.dram_tensor("bs_b", [128, ME], F32, kind="ExternalInput")
    bs_d = nc.dram_tensor("bs_d", [128, MD], F32, kind="ExternalInput")
    b1a = nc.dram_tensor("b1a", [128, KD], F32, kind="ExternalInput")
    b1h = nc.dram_tensor("b1h", [128, KD], F32, kind="ExternalInput")

    # ---- outputs ----
    out = nc.dram_tensor("out", [B, T, VS], F32, kind="ExternalOutput")

    # ---- DRAM scratch ----
    xf_d = nc.dram_tensor("xf_d", [S, 128, ME * 32], BF16)
    xb_d = nc.dram_tensor("xb_d", [S, 128, ME * 32], BF16)
    xd_d = nc.dram_tensor("xd_d", [T, 128, MD * 32], BF16)

    with tile.TileContext(nc) as tc, ExitStack() as ctx:
        wpool = ctx.enter_context(tc.tile_pool(name="weights", bufs=1))
        spool = ctx.enter_context(tc.tile_pool(name="state", bufs=1))
        big = ctx.enter_context(tc.tile_pool(name="big", bufs=1))

        # ---------- load weights / constants ----------
        def load(dram, shape, dtype, tag):
            t = wpool.tile(shape, dtype, tag=tag, name=tag)
            nc.sync.dma_start(t[:], dram[:, :])
            return t

        wih_f_s = load(wih_f, [128, ME * 128], BF16, "wih_f")
        wih_b_s = load(wih_b, [128, ME * 128], BF16, "wih_b")
        wih_d_s = load(wih_d, [128, MD * 128], BF16, "wih_d")
        whh_f_s = load(whh_f, [128, KE * ME * 128], BF16, "whh_f")
        whh_b_s = load(whh_b, [128, KE * ME * 128], BF16, "whh_b")
        whh_d_s = load(whh_d, [128, KD * MD * 128], BF16, "whh_d")
        wtr_s = load(wtr, [128, KD * KD * 128], BF16, "wtr")
        wf1_s = load(wf1, [128, 8 * KD * 128], BF16, "wf1")
        bs_f_s = load(bs_f, [128, ME], F32, "bs_f")
        bs_b_s = load(bs_b, [128, ME], F32, "bs_b")
        bs_d_s = load(bs_d, [128, MD], F32, "bs_d")
        b1a_s = load(b1a, [128, KD], F32, "b1a")
        b1h_s = load(b1h, [128, KD], F32, "b1h")
        startT_s = load(startT, [128, 1], BF16, "startT")
        e0T_s = load(e0T, [128, KD], F32, "e0T")
        s0T_s = load(s0T, [128, KD], F32, "s0T")
        ident = wpool.tile([128, 128], BF16, tag="ident", name="ident")
        make_identity(nc, ident)

        diff_e_s = wpool.tile([1, 2 * D_ENC], BF16, tag="diff_e", name="diff_e")
        nc.sync.dma_start(diff_e_s[:], diff_e[:, :])
        diff_s_s = wpool.tile([1, D_DEC], BF16, tag="diff_s", name="diff_s")
        nc.sync.dma_start(diff_s_s[:], diff_s[:, :])
        lab_i_s = wpool.tile([1, B], BF16, tag="lab_i", name="lab_i")
        nc.sync.dma_start(lab_i_s[:], lab_i[:, :])
        lab_d_s = wpool.tile([1, B], BF16, tag="lab_d", name="lab_d")
        nc.sync.dma_start(lab_d_s[:], lab_d[:, :])

        # ---------- gathers ----------
        idx_e_s = wpool.tile([128, NI_E // 16], I16, tag="idx_e", name="idx_e")
        nc.sync.dma_start(idx_e_s[:], idx_e[:, :])
        idx_d_s = wpool.tile([128, NI_D // 16], I16, tag="idx_d", name="idx_d")
        nc.sync.dma_start(idx_d_s[:], idx_d[:, :])
        encT = big.tile([128, 1, NI_E], BF16, tag="encT", name="encT")   # cols s*32+b
        nc.gpsimd.dma_gather(encT[:], tokb[:, :], idx_e_s[:], NI_E, NI_E,
                             D_EMB, transpose=True, single_packet=False)
        decT = big.tile([128, 1, NI_D], BF16, tag="decT", name="decT")   # cols t*32+b
        nc.gpsimd.dma_gather(decT[:], tokb[:, :], idx_d_s[:], NI_D, NI_D,
                             D_EMB, transpose=True, single_packet=False)
        # decoder step 0 input = start_emb (broadcast over b)
        nc.vector.tensor_copy(decT[:, 0, 0:B],
                              startT_s[:, 0:1].to_broadcast((128, B)))

        # ---------- input projections -> DRAM ----------
        with tc.tile_pool(name="xp_ps", bufs=4, space="PSUM") as xps, \
             tc.tile_pool(name="xp_sb", bufs=4) as xsb:
            def proj(w_s, src, nblk, nm, bias_s, dst, L):
                # dst [L, 128, nm*32]; block covers 512/B steps
                spb = 512 // B
                for blk in range(nblk):
                    for m in range(nm):
                        ps = xps.tile([128, 512], F32, tag="xp", name="xp")
                        nc.tensor.matmul(ps[:], w_s[:, m * 128:(m + 1) * 128],
                                         src[:, 0, blk * 512:(blk + 1) * 512],
                                         start=True, stop=True)
                        sb = xsb.tile([128, 512], BF16, tag="xs", name="xs")
                        nc.scalar.activation(sb[:], ps[:], AF.Identity,
                                             bias=bias_s[:, m:m + 1])
                        dst_ap = dst.ap()[blk * spb:(blk + 1) * spb, :,
                                          m * 32:(m + 1) * 32]
                        nc.sync.dma_start(
                            dst_ap.rearrange("s p b -> p s b"),
                            sb[:].rearrange("p (s b) -> p s b", b=B))

            proj(wih_f_s, encT, NBLK_E, ME, bs_f_s, xf_d, S)
            proj(wih_b_s, encT, NBLK_E, ME, bs_b_s, xb_d, S)
            proj(wih_d_s, decT, NBLK_D, MD, bs_d_s, xd_d, T)

        try:
            if phases < 2:
                raise _Stop
            # ---------- init states ----------
            # encoder h0 per direction via outer product + broadcast add
            h_f = [spool.tile([128, KE * B], BF16, tag=f"h_f{j}", name=f"h_f{j}") for j in range(2)]
            c_f = spool.tile([128, KE * B], F32, tag="c_f", name="c_f")
            h_b = [spool.tile([128, KE * B], BF16, tag=f"h_b{j}", name=f"h_b{j}") for j in range(2)]
            c_b = spool.tile([128, KE * B], F32, tag="c_b", name="c_b")
            mem_T = big.tile([128, 2 * KE * B * S], BF16, tag="mem_T", name="mem_T")  # cols k*(B*S)+b*S+s

            with tc.tile_pool(name="init_ps", bufs=2, space="PSUM") as ips:
                for d, (hst, dbase) in enumerate([(h_f, 0), (h_b, KE)]):
                    for k in range(KE):
                        ps = ips.tile([128, B], F32, tag="i", name="i")
                        col = (dbase + k) * 128
                        nc.tensor.matmul(ps[:], diff_e_s[:, col:col + 128],
                                         lab_i_s[:, :], start=True, stop=True)
                        f32t = spool.tile([128, B], F32, tag="h0f", name="h0f")
                        nc.vector.tensor_scalar_add(f32t[:], ps[:],
                                                    e0T_s[:, dbase + k:dbase + k + 1])
                        nc.vector.tensor_copy(hst[0][:, k * B:(k + 1) * B], f32t[:])
                nc.vector.memset(c_f[:], 0.0)
                nc.vector.memset(c_b[:], 0.0)

            # ---------- encoder recurrence (both directions interleaved) ----------
            memT4 = mem_T[:].rearrange("p (k b s) -> p k b s", k=2 * KE, b=B)
            with tc.tile_pool(name="enc_ps", bufs=2, space="PSUM") as eps, \
                 tc.tile_pool(name="enc_x", bufs=4) as exp_, \
                 tc.tile_pool(name="enc_g", bufs=3) as egp:
                for step in range(S):
                    for d, (hst, cst, whh_s, xdram, bs_s, kk0) in enumerate([
                            (h_f, c_f, whh_f_s, xf_d, bs_f_s, 0),
                            (h_b, c_b, whh_b_s, xb_d, bs_b_s, KE)]):
                        s_in = step if d == 0 else S - 1 - step
                        s_mem = s_in
                        hcur = hst[step % 2]
                        hnxt = hst[(step + 1) % 2]
                        xt = exp_.tile([128, ME * 32], BF16, tag=f"x{d}", name=f"x{d}")
                        nc.sync.dma_start(xt[:], xdram[s_in, :, :])
                        ps = eps.tile([128, ME * 32], F32, tag=f"g{d}", name=f"g{d}")
                        for m in range(ME):
                            for k in range(KE):
                                rhs = hcur[:, k * B:(k + 1) * B]
                                lt = whh_s[:, (k * ME + m) * 128:(k * ME + m + 1) * 128]
                                nc.tensor.matmul(ps[:, m * 32:(m + 1) * 32], lt, rhs,
                                                 start=(k == 0), stop=(k == KE - 1))
                        g = egp.tile([128, ME * 32], F32, tag=f"gg{d}", name=f"gg{d}")
                        nc.vector.tensor_tensor(g[:], ps[:], xt[:], ALU.add)
                        # gates: i [0:2B*KE], f, g, o   (ME*32 = 4*KE*32)
                        GW = KE * 32  # width of one gate group
                        nc.scalar.activation(g[:, 0:2 * GW], g[:, 0:2 * GW], AF.Sigmoid)
                        nc.scalar.activation(g[:, 2 * GW:3 * GW], g[:, 2 * GW:3 * GW],
                                             AF.Tanh)
                        nc.scalar.activation(g[:, 3 * GW:4 * GW], g[:, 3 * GW:4 * GW],
                                             AF.Sigmoid)
                        t1 = egp.tile([128, GW], F32, tag=f"t1{d}", name=f"t1{d}")
                        nc.vector.tensor_tensor(t1[:], g[:, GW:2 * GW], cst[:], ALU.mult)
                        t2 = egp.tile([128, GW], F32, tag=f"t2{d}", name=f"t2{d}")
                        nc.vector.tensor_tensor(t2[:], g[:, 0:GW], g[:, 2 * GW:3 * GW],
                                                ALU.mult)
                        nc.vector.tensor_tensor(cst[:], t1[:], t2[:], ALU.add)
                        tc_t = egp.tile([128, GW], F32, tag=f"tc{d}", name=f"tc{d}")
                        nc.scalar.activation(tc_t[:], cst[:], AF.Tanh)
                        nc.vector.tensor_tensor(hnxt[:], g[:, 3 * GW:4 * GW], tc_t[:],
                                                ALU.mult)
                        # store h into mem_T (cols (kk0+k)*B*S + b*S + s_mem)
                        nc.vector.tensor_copy(
                            memT4[:, kk0:kk0 + KE, :, s_mem],
                            hnxt[:].rearrange("p (k b) -> p k b", k=KE))

            if phases < 3:
                raise _Stop
            # ---------- c_t / h_t decoder init ----------
            h_d = [spool.tile([128, KD * B], BF16, tag=f"h_d{j}", name=f"h_d{j}") for j in range(2)]
            c_d = spool.tile([128, KD * B], F32, tag="c_d", name="c_d")
            # ccT = [cf.T; cb.T] as bf16 rhs chunks [128, B] (k = 0..KD-1)
            ccT = spool.tile([128, KD * B], BF16, tag="ccT", name="ccT")
            nc.vector.tensor_copy(ccT[:, 0:KE * B], c_f[:])
            nc.vector.tensor_copy(ccT[:, KE * B:2 * KE * B], c_b[:])
            with tc.tile_pool(name="ct_ps", bufs=2, space="PSUM") as cps, \
                 tc.tile_pool(name="ct_sb", bufs=2) as csb:
                for m in range(KD):
                    ps = cps.tile([128, B], F32, tag="ct", name="ct")
                    for k in range(KD):
                        lt = wtr_s[:, (k * KD + m) * 128:(k * KD + m + 1) * 128]
                        nc.tensor.matmul(ps[:], lt, ccT[:, k * B:(k + 1) * B],
                                         start=(k == 0), stop=(k == KD - 1))
                    # lrelu(y) = 0.55 y + 0.45 |y|
                    ab = csb.tile([128, B], F32, tag="ab", name="ab")
                    nc.scalar.activation(ab[:], ps[:], AF.Abs)
                    ident_t = csb.tile([128, B], F32, tag="idt", name="idt")
                    nc.scalar.activation(ident_t[:], ps[:], AF.Identity, scale=0.55)
                    nc.vector.scalar_tensor_tensor(c_d[:, m * B:(m + 1) * B], ab[:],
                                                   0.45, ident_t[:], ALU.mult, ALU.add)
                # h_t = s0 + lab*(s1-s0)
                for k in range(KD):
                    ps = cps.tile([128, B], F32, tag="ct", name="ct")
                    nc.tensor.matmul(ps[:], diff_s_s[:, k * 128:(k + 1) * 128],
                                     lab_d_s[:, :], start=True, stop=True)
                    f32t = csb.tile([128, B], F32, tag="h0d", name="h0d")
                    nc.vector.tensor_scalar_add(f32t[:], ps[:], s0T_s[:, k:k + 1])
                    nc.vector.tensor_copy(h_d[0][:, k * B:(k + 1) * B], f32t[:])

            if phases < 4:
                raise _Stop
            # ---------- decoder recurrence ----------
            H_T = big.tile([128, KD * B * T], BF16, tag="H_T", name="H_T")  # cols k*(B*T)+b*T+t
            HT4 = H_T[:].rearrange("p (k b t) -> p k b t", k=KD, b=B)
            with tc.tile_pool(name="dec_ps", bufs=2, space="PSUM") as dps, \
                 tc.tile_pool(name="dec_x", bufs=4) as dxp, \
                 tc.tile_pool(name="dec_g", bufs=3) as dgp:
                GW = KD * 32
                for step in range(T):
                    hcur = h_d[step % 2]
                    hnxt = h_d[(step + 1) % 2]
                    xt = dxp.tile([128, MD * 32], BF16, tag="xd", name="xd")
                    nc.sync.dma_start(xt[:], xd_d[step, :, :])
                    ps = dps.tile([128, MD * 32], F32, tag="gd", name="gd")
                    for m in range(MD):
                        for k in range(KD):
                            rhs = hcur[:, k * B:(k + 1) * B]
                            lt = whh_d_s[:, (k * MD + m) * 128:(k * MD + m + 1) * 128]
                            nc.tensor.matmul(ps[:, m * 32:(m + 1) * 32], lt, rhs,
                                             start=(k == 0), stop=(k == KD - 1))
                    g = dgp.tile([128, MD * 32], F32, tag="ggd", name="ggd")
                    nc.vector.tensor_tensor(g[:], ps[:], xt[:], ALU.add)
                    nc.scalar.activation(g[:, 0:2 * GW], g[:, 0:2 * GW], AF.Sigmoid)
                    nc.scalar.activation(g[:, 2 * GW:3 * GW], g[:, 2 * GW:3 * GW], AF.Tanh)
                    nc.scalar.activation(g[:, 3 * GW:4 * GW], g[:, 3 * GW:4 * GW],
                                         AF.Sigmoid)
                    t1 = dgp.tile([128, GW], F32, tag="t1d", name="t1d")
                    nc.vector.tensor_tensor(t1[:], g[:, GW:2 * GW], c_d[:], ALU.mult)
                    t2 = dgp.tile([128, GW], F32, tag="t2d", name="t2d")
                    nc.vector.tensor_tensor(t2[:], g[:, 0:GW], g[:, 2 * GW:3 * GW],
                                            ALU.mult)
                    nc.vector.tensor_tensor(c_d[:], t1[:], t2[:], ALU.add)
                    tc_t = dgp.tile([128, GW], F32, tag="tcd", name="tcd")
                    nc.scalar.activation(tc_t[:], c_d[:], AF.Tanh)
                    nc.vector.tensor_tensor(hnxt[:], g[:, 3 * GW:4 * GW], tc_t[:],
                                            ALU.mult)
                    nc.vector.tensor_copy(
                        HT4[:, :, :, step],
                        hnxt[:].rearrange("p (k b) -> p k b", k=KD))

            if phases < 5:
                raise _Stop
            # ---------- attention + FFN mid ----------
            mid_T = big.tile([128, KD * B * T], BF16, tag="mid_T", name="mid_T")  # cols k*(B*T)+bt
            BT = B * T
            n_mblk = BT // 512
            bpb = 512 // T if T <= 512 else 1  # b's per 512-col block
            with tc.tile_pool(name="at_ps", bufs=2, space="PSUM") as aps, \
                 tc.tile_pool(name="pt_ps", bufs=2, space="PSUM") as pps, \
                 tc.tile_pool(name="cx_ps", bufs=2, space="PSUM") as cps2, \
                 tc.tile_pool(name="md_ps", bufs=2, space="PSUM") as mps, \
                 tc.tile_pool(name="at_sb", bufs=3) as asb, \
                 tc.tile_pool(name="cx_sb", bufs=2) as cxs, \
                 tc.tile_pool(name="mn_sb", bufs=2) as mns:
                for blk in range(n_mblk):
                    ctx_blk = cxs.tile([128, KD, 512], BF16, tag="cxb", name="cxb")
                    for bi in range(bpb):
                        b = blk * bpb + bi
                        # scores A_b [T, S]
                        a_ps = aps.tile([T, S], F32, tag="a", name="a")
                        for k in range(2 * KE):
                            nc.tensor.matmul(a_ps[:], HT4[:, k, b, :],
                                             memT4[:, k, b, :],
                                             start=(k == 0), stop=(k == 2 * KE - 1))
                        mx = asb.tile([T, 1], F32, tag="mx", name="mx")
                        nc.vector.tensor_reduce(mx[:], a_ps[:], AX.X, ALU.max,
                                                negate=True)
                        mx2 = asb.tile([T, 1], F32, tag="mx2", name="mx2")
                        nc.scalar.mul(mx2[:], mx[:], SCALE)
                        ex = asb.tile([T, S], F32, tag="ex", name="ex")
                        den = asb.tile([T, 1], F32, tag="den", name="den")
                        nc.scalar.activation(ex[:], a_ps[:], AF.Exp, bias=mx2[:],
                                             scale=SCALE, accum_out=den[:])
                        rec = asb.tile([T, 1], F32, tag="rec", name="rec")
                        nc.vector.reciprocal(rec[:], den[:])
                        p_sb = asb.tile([T, S], BF16, tag="p", name="p")
                        nc.vector.tensor_scalar_mul(p_sb[:], ex[:], rec[:])
                        # P.T [S, T]
                        pt_ps = pps.tile([S, T], BF16, tag="pt", name="pt", padded_shape=[128, 128])
                        nc.tensor.transpose(pt_ps[:], p_sb[:], ident[0:T, 0:T])
                        pt_sb = asb.tile([S, T], BF16, tag="pts", name="pts")
                        nc.scalar.copy(pt_sb[:], pt_ps[:])
                        # mem_norm(b) [s, d] built on the fly by PE transpose
                        memNb = mns.tile([S, 2 * KE, 128], BF16, tag="mnb", name="mnb")
                        for kd in range(2 * KE):
                            mn_ps = pps.tile([S, 128], BF16, tag="pt", name="mnp", padded_shape=[128, 128])
                            nc.tensor.transpose(mn_ps[:], memT4[:, kd, b, :], ident[:])
                            nc.scalar.copy(memNb[:, kd, :], mn_ps[:])
                        # ctx.T chunks [128, T] = mem_norm(b,k).T @ P.T
                        for kd in range(KD):
                            c_ps = cps2.tile([128, T], F32, tag="c", name="c")
                            nc.tensor.matmul(c_ps[:], memNb[:, kd, :], pt_sb[:],
                                             start=True, stop=True)
                            nc.vector.tensor_copy(
                                ctx_blk[:, kd, bi * T:(bi + 1) * T], c_ps[:])
                    # mid.T [512-rows, this 512-col block]
                    for m in range(KD):
                        ps = mps.tile([128, 512], F32, tag="md", name="md")
                        for k in range(KD):  # h part
                            lt = wf1_s[:, (k * KD + m) * 128:(k * KD + m + 1) * 128]
                            nc.tensor.matmul(ps[:], lt,
                                             H_T[:, k * BT + blk * 512:
                                                 k * BT + (blk + 1) * 512],
                                             start=(k == 0), stop=False)
                        for k in range(KD):  # ctx part
                            kk = KD + k
                            lt = wf1_s[:, (kk * KD + m) * 128:(kk * KD + m + 1) * 128]
                            nc.tensor.matmul(ps[:], lt, ctx_blk[:, k, :],
                                             start=False, stop=(k == KD - 1))
                        ab = asb.tile([128, 512], F32, tag="mab", name="mab")
                        nc.scalar.activation(ab[:], ps[:], AF.Abs,
                                             bias=b1a_s[:, m:m + 1])
                        idt = asb.tile([128, 512], F32, tag="mid", name="mid")
                        nc.scalar.activation(idt[:], ps[:], AF.Identity, scale=0.55,
                                             bias=b1h_s[:, m:m + 1])
                        nc.vector.scalar_tensor_tensor(
                            mid_T[:, m * BT + blk * 512:m * BT + (blk + 1) * 512],
                            ab[:], 0.45, idt[:], ALU.mult, ALU.add)

            if phases < 6:
                raise _Stop
            # ---------- vocab projection ----------
            wf2_3d = wf2.ap().rearrange("p (k v) -> p k v", k=KD)
            with tc.tile_pool(name="lg_ps", bufs=2, space="PSUM") as lps, \
                 tc.tile_pool(name="lg_sb", bufs=3) as lsb, \
                 tc.tile_pool(name="wf2_sb", bufs=2) as wfp:
                for c in range(NVC):
                    w0 = c * 500
                    w1 = min(VS, w0 + 500)
                    nw = w1 - w0
                    wf2c = wfp.tile([128, KD, 500], BF16, tag="wf2c", name="wf2c")
                    nc.sync.dma_start(wf2c[:, :, 0:nw], wf2_3d[:, :, w0:w1])
                    for btm in range(BT // 128):
                        ps = lps.tile([128, 500], F32, tag="lg", name="lg")
                        for k in range(KD):
                            lt = mid_T[:, k * BT + btm * 128:k * BT + (btm + 1) * 128]
                            nc.tensor.matmul(ps[:, 0:nw], lt,
                                             wf2c[:, k, 0:nw],
                                             start=(k == 0), stop=(k == KD - 1))
                        sb = lsb.tile([128, 500], F32, tag="lo", name="lo")
                        nc.scalar.copy(sb[:, 0:nw], ps[:, 0:nw])
                        # btm-th 128 bt-cols are b = btm*128//T .. : with b-major
                        # cols b*T+t, a 128-col chunk spans 128/T b's (T>=128: b=btm)
                        if T >= 128:
                            b0 = (btm * 128) // T
                            t0 = (btm * 128) % T
                            nc.sync.dma_start(out.ap()[b0, t0:t0 + 128, w0:w1],
                                              sb[:, 0:nw])
                        else:
                            nb = 128 // T
                            b0 = btm * nb
                            nc.sync.dma_start(
                                out.ap()[b0:b0 + nb, :, w0:w1]
                                .rearrange("b t v -> (b t) v"),
                                sb[:, 0:nw])

        except _Stop:
            pass
    nc.compile()
    return nc


def prep_inputs(i, S=128, T=128, V=32000, VS=4000):
    """Host-side input staging -> list of 8 per-core in_maps."""
    KD = D_DEC // 128
    ME = 4 * D_ENC // 128
    MD = 4 * D_DEC // 128
    KE = D_ENC // 128

    def as_np(x, dt=np.float32):
        return np.ascontiguousarray(np.asarray(x), dtype=dt)

    tok = as_np(i["tok_emb"]).astype(bf16)

    def idx_prep(flat):
        a = flat.astype(np.int16).reshape(-1, 16).T  # [16, N/16]
        return np.ascontiguousarray(np.tile(a, (8, 1)))

    inp = as_np(i["inp"], np.int64)
    x = as_np(i["x"], np.int64)
    idx_e = idx_prep(inp.T.reshape(-1))              # s-major: s*32+b
    dmat = np.zeros((B, T), np.int64)
    dmat[:, 1:] = x[:, :T - 1]
    idx_d = idx_prep(dmat.T.reshape(-1))             # t-major: t*32+b

    startT = as_np(i["start_emb"]).reshape(D_EMB, 1).astype(bf16)

    est = as_np(i["enc_style_emb"])                  # [2, 512]
    diff_e = (est[1] - est[0]).reshape(1, -1).astype(bf16)
    e0T = np.ascontiguousarray(est[0].reshape(KD, 128).T)  # [128, 4]
    sty = as_np(i["style_emb"])                      # [2, 512]
    diff_s = (sty[1] - sty[0]).reshape(1, -1).astype(bf16)
    s0T = np.ascontiguousarray(sty[0].reshape(KD, 128).T)
    lab_i = as_np(i["label_i"], np.float32).reshape(1, B).astype(bf16)
    lab_d = as_np(i["label"], np.float32).reshape(1, B).astype(bf16)

    def wihT(w, nm):   # w [4H, 128] -> [128, nm*128]
        a = w.reshape(nm, 128, 128)        # [m, c, p]
        return np.ascontiguousarray(a.transpose(2, 0, 1).reshape(128, nm * 128)
                                    ).astype(bf16)

    def whhT(w, nk, nm):  # w [4H, H] -> [128, nk*nm*128], (k,m)-major
        a = w.reshape(nm, 128, nk, 128)    # [m, c, k, p]
        a = a.transpose(3, 2, 0, 1)        # [p, k, m, c]
        return np.ascontiguousarray(a.reshape(128, nk * nm * 128)).astype(bf16)

    wih_f = wihT(as_np(i["Wih_f"]), ME)
    wih_b = wihT(as_np(i["Wih_b"]), ME)
    wih_d = wihT(as_np(i["Wih_d"]), MD)
    whh_f = whhT(as_np(i["Whh_f"]), KE, ME)
    whh_b = whhT(as_np(i["Whh_b"]), KE, ME)
    whh_d = whhT(as_np(i["Whh_d"]), KD, MD)
    wtr = whhT(as_np(i["W_tr"]), KD, KD)
    wf1 = whhT(as_np(i["W_f1"]), 8, KD)

    wf2_full = as_np(i["W_f2"])                      # [V, 512]

    def bcol(v, nm):
        return np.ascontiguousarray(v.reshape(nm, 128).T)  # [128, nm]

    bs_f = bcol(as_np(i["bih_f"]) + as_np(i["bhh_f"]), ME)
    bs_b = bcol(as_np(i["bih_b"]) + as_np(i["bhh_b"]), ME)
    bs_d = bcol(as_np(i["bih_d"]) + as_np(i["bhh_d"]), MD)
    b1 = as_np(i["b_f1"])
    b1a = bcol(b1, KD)
    b1h = bcol(0.55 * b1, KD)

    common = dict(tokb=tok, idx_e=idx_e, idx_d=idx_d, startT=startT,
                  diff_e=diff_e, e0T=e0T, lab_i=lab_i,
                  diff_s=diff_s, s0T=s0T, lab_d=lab_d,
                  wih_f=wih_f, wih_b=wih_b, wih_d=wih_d,
                  whh_f=whh_f, whh_b=whh_b, whh_d=whh_d,
                  wtr=wtr, wf1=wf1,
                  bs_f=bs_f, bs_b=bs_b, bs_d=bs_d, b1a=b1a, b1h=b1h)
    in_maps = []
    for c in range(N_CORES):
        shard = wf2_full[c * VS:(c + 1) * VS]        # [VS, 512]
        a = shard.reshape(VS, KD, 128)               # [v, k, p]
        wf2 = np.ascontiguousarray(a.transpose(2, 1, 0).reshape(128, KD * VS)
                                   ).astype(bf16)
        in_maps.append(dict(common, wf2=wf2))
    return in_maps


_NC_CACHE = {}


def kernel(**inputs):
    key = "full"
    if key not in _NC_CACHE:
        _NC_CACHE[key] = build()
    nc = _NC_CACHE[key]
    in_maps = prep_inputs(inputs)
    res = run_bass_kernel_spmd(nc, in_maps, core_ids=list(range(N_CORES)))
    return np.concatenate([r["out"] for r in res.results], axis=2)



# revision 5
# speedup vs baseline: 1.3332x; 1.3332x over previous
"""DenoiseLSTM Trainium2 kernel (8 NeuronCores, SPMD) — v2.

Strategy vs v1 baseline:
- Input projections are folded into the recurrent loops as extra matmul
  accumulation steps (embeddings live in SBUF from the gather), killing the
  DMA-scatter projection phase and the per-step g=ps+xt vector add.
- Gate m-chunks are permuted host-side to [i, f, o, g] so one sigmoid
  activation covers i/f/o straight out of PSUM; tanh covers g.
- Attention + FFN + vocab projection are restructured per 32-step t-block
  and emitted interleaved with decoder steps, so the Tile scheduler fills
  the decoder's dependency-stall gaps with post-work.
- Attention scores pack 4 batch rows into one [128,S] PSUM tile via
  tile_position column packing; softmax runs on full 128 partitions.
- Output is written bf16 and upcast on host (halves output DMA).
- Optional: recurrent Whh weights in fp8-e4m3 (halves LDWEIGHTS traffic);
  CPU-simulated rel err 0.0079 vs 2e-2 budget.

Replicated recurrences on all 8 cores; vocab projection sharded over V
(4000 columns per core); host concatenates V-shards.
"""
import sys

sys.path.insert(0, "/opt/trn_rl_repo")

from contextlib import ExitStack

import numpy as np
import ml_dtypes

import concourse.bass as bass
import concourse.bacc as bacc
import concourse.mybir as mybir
import concourse.tile as tile
from concourse.bass_utils import run_bass_kernel_spmd
from concourse.masks import make_identity

bf16 = ml_dtypes.bfloat16
f8e4 = ml_dtypes.float8_e4m3
F32 = mybir.dt.float32
BF16 = mybir.dt.bfloat16
FP8E4 = mybir.dt.float8e4
I16 = mybir.dt.int16
AF = mybir.ActivationFunctionType
ALU = mybir.AluOpType
AX = mybir.AxisListType

B = 32
D_EMB = 128
D_ENC = 256
D_DEC = 512
N_CORES = 8
KE = D_ENC // 128   # 2
KD = D_DEC // 128   # 4
ME = 4 * D_ENC // 128  # 8 gate chunks (encoder)
MD = 4 * D_DEC // 128  # 16 gate chunks (decoder)
# m-chunk permutation: [i-chunks, f-chunks, o-chunks, g-chunks]
PERM_E = [0, 1, 2, 3, 6, 7, 4, 5]
PERM_D = [0, 1, 2, 3, 4, 5, 6, 7, 12, 13, 14, 15, 8, 9, 10, 11]


class _Stop(Exception):
    pass


def build(S=128, T=128, V=32000, VS=4000, phases=6, fp8=False, bias_mode=False):
    NI_E = B * S
    NI_D = B * T
    Tc = 32                      # timesteps per post-work block
    NBLK = T // Tc               # 4
    BG = 4                       # batch rows packed per score tile
    SCALE = 1.0 / float(np.sqrt(np.float32(2 * D_ENC)))
    WHH_DT = FP8E4 if fp8 else BF16

    nc = bacc.Bacc("TRN2", target_bir_lowering=False, debug=False)

    # ---- external inputs ----
    tokb = nc.dram_tensor("tokb", [V, D_EMB], BF16, kind="ExternalInput")
    idx_e = nc.dram_tensor("idx_e", [128, NI_E // 16], I16, kind="ExternalInput")
    idx_d = nc.dram_tensor("idx_d", [128, NI_D // 16], I16, kind="ExternalInput")
    startT = nc.dram_tensor("startT", [128, 1], BF16, kind="ExternalInput")
    diff_e = nc.dram_tensor("diff_e", [1, 2 * D_ENC], BF16, kind="ExternalInput")
    e0T = nc.dram_tensor("e0T", [128, KD], F32, kind="ExternalInput")
    lab_i = nc.dram_tensor("lab_i", [1, B], BF16, kind="ExternalInput")
    diff_s = nc.dram_tensor("diff_s", [1, D_DEC], BF16, kind="ExternalInput")
    s0T = nc.dram_tensor("s0T", [128, KD], F32, kind="ExternalInput")
    lab_d = nc.dram_tensor("lab_d", [1, B], BF16, kind="ExternalInput")
    # input-projection weights (bf16), m-perm-major
    wih_f = nc.dram_tensor("wih_f", [128, ME * 128], BF16, kind="ExternalInput")
    wih_b = nc.dram_tensor("wih_b", [128, ME * 128], BF16, kind="ExternalInput")
    wih_d = nc.dram_tensor("wih_d", [128, MD * 128], BF16, kind="ExternalInput")
    # recurrent weights, (m-perm, k)-major, optionally fp8
    whh_f = nc.dram_tensor("whh_f", [128, ME * KE * 128], WHH_DT, kind="ExternalInput")
    whh_b = nc.dram_tensor("whh_b", [128, ME * KE * 128], WHH_DT, kind="ExternalInput")
    whh_d = nc.dram_tensor("whh_d", [128, MD * KD * 128], WHH_DT, kind="ExternalInput")
    wtr = nc.dram_tensor("wtr", [128, KD * KD * 128], BF16, kind="ExternalInput")
    wf1 = nc.dram_tensor("wf1", [128, 8 * KD * 128], BF16, kind="ExternalInput")
    wf2 = nc.dram_tensor("wf2", [128, KD * VS], BF16, kind="ExternalInput")
    b1c = nc.dram_tensor("b1c", [128, KD], F32, kind="ExternalInput")
    if bias_mode:
        brow_f = nc.dram_tensor("brow_f", [1, ME * 128], BF16, kind="ExternalInput")
        brow_b = nc.dram_tensor("brow_b", [1, ME * 128], BF16, kind="ExternalInput")
        brow_d = nc.dram_tensor("brow_d", [1, MD * 128], BF16, kind="ExternalInput")

    out = nc.dram_tensor("out", [B, T, VS], BF16, kind="ExternalOutput")

    with tile.TileContext(nc) as tc, ExitStack() as ctx:
        wpool = ctx.enter_context(tc.tile_pool(name="weights", bufs=1))
        spool = ctx.enter_context(tc.tile_pool(name="state", bufs=1))
        big = ctx.enter_context(tc.tile_pool(name="big", bufs=1))

        def load(dram, shape, dtype, tag):
            t = wpool.tile(shape, dtype, tag=tag, name=tag)
            nc.sync.dma_start(t[:], dram[:, :])
            return t

        wih_f_s = load(wih_f, [128, ME * 128], BF16, "wih_f")
        wih_b_s = load(wih_b, [128, ME * 128], BF16, "wih_b")
        wih_d_s = load(wih_d, [128, MD * 128], BF16, "wih_d")
        whh_f_s = load(whh_f, [128, ME * KE * 128], WHH_DT, "whh_f")
        whh_b_s = load(whh_b, [128, ME * KE * 128], WHH_DT, "whh_b")
        whh_d_s = load(whh_d, [128, MD * KD * 128], WHH_DT, "whh_d")
        wtr_s = load(wtr, [128, KD * KD * 128], BF16, "wtr")
        wf1_s = load(wf1, [128, 8 * KD * 128], BF16, "wf1")
        wf2_s = load(wf2, [128, KD * VS], BF16, "wf2")
        b1c_s = load(b1c, [128, KD], F32, "b1c")
        e0T_s = load(e0T, [128, KD], F32, "e0T")
        s0T_s = load(s0T, [128, KD], F32, "s0T")
        startT_s = load(startT, [128, 1], BF16, "startT")
        ident = wpool.tile([128, 128], BF16, tag="ident", name="ident")
        make_identity(nc, ident)

        diff_e_s = wpool.tile([1, 2 * D_ENC], BF16, tag="diff_e", name="diff_e")
        nc.sync.dma_start(diff_e_s[:], diff_e[:, :])
        diff_s_s = wpool.tile([1, D_DEC], BF16, tag="diff_s", name="diff_s")
        nc.sync.dma_start(diff_s_s[:], diff_s[:, :])
        lab_i_s = wpool.tile([1, B], BF16, tag="lab_i", name="lab_i")
        nc.sync.dma_start(lab_i_s[:], lab_i[:, :])
        lab_d_s = wpool.tile([1, B], BF16, tag="lab_d", name="lab_d")
        nc.sync.dma_start(lab_d_s[:], lab_d[:, :])
        if bias_mode:
            brow_f_s = load(brow_f, [1, ME * 128], BF16, "brow_f")
            brow_b_s = load(brow_b, [1, ME * 128], BF16, "brow_b")
            brow_d_s = load(brow_d, [1, MD * 128], BF16, "brow_d")
            ones_s = wpool.tile([1, B], BF16, tag="ones", name="ones")
            nc.vector.memset(ones_s[:], 1.0)

        # ---------- gathers ----------
        idx_e_s = wpool.tile([128, NI_E // 16], I16, tag="idx_e", name="idx_e")
        nc.sync.dma_start(idx_e_s[:], idx_e[:, :])
        idx_d_s = wpool.tile([128, NI_D // 16], I16, tag="idx_d", name="idx_d")
        nc.sync.dma_start(idx_d_s[:], idx_d[:, :])
        decT = big.tile([128, 1, NI_D], BF16, tag="decT", name="decT")  # cols t*32+b
        nc.gpsimd.dma_gather(decT[:], tokb[:, :], idx_d_s[:], NI_D, NI_D,
                             D_EMB, transpose=True, single_packet=False)
        nc.vector.tensor_copy(decT[:, 0, 0:B],
                              startT_s[:, 0:1].to_broadcast((128, B)))

        # ---------- shared state ----------
        mem_T = big.tile([128, 2 * KE * B * S], BF16, tag="mem_T", name="mem_T")
        memT4 = mem_T[:].rearrange("p (k b s) -> p k b s", k=2 * KE, b=B)
        memN = big.tile([128, B * 2 * KE * 128], BF16, tag="memN", name="memN")

        h_f = [spool.tile([128, KE * B], BF16, tag=f"h_f{j}", name=f"h_f{j}")
               for j in range(2)]
        h_b = [spool.tile([128, KE * B], BF16, tag=f"h_b{j}", name=f"h_b{j}")
               for j in range(2)]
        c_f = spool.tile([128, KE * B], F32, tag="c_f", name="c_f")
        c_b = spool.tile([128, KE * B], F32, tag="c_b", name="c_b")

        try:
            # ============ encoder ============
            with tc.tile_pool(name="encT_pool", bufs=1) as encp:
                encT = encp.tile([128, 1, NI_E], BF16, tag="encT", name="encT")
                nc.gpsimd.dma_gather(encT[:], tokb[:, :], idx_e_s[:], NI_E, NI_E,
                                     D_EMB, transpose=True, single_packet=False)

                # h0 init via outer product + broadcast add
                with tc.tile_pool(name="init_ps", bufs=2, space="PSUM") as ips, \
                     tc.tile_pool(name="init_sb", bufs=2) as isb:
                    for d, (hst, dbase) in enumerate([(h_f, 0), (h_b, KE)]):
                        for k in range(KE):
                            ps = ips.tile([128, B], F32, tag="i", name="i")
                            col = (dbase + k) * 128
                            nc.tensor.matmul(ps[:], diff_e_s[:, col:col + 128],
                                             lab_i_s[:, :], start=True, stop=True)
                            f32t = isb.tile([128, B], F32, tag="h0f", name="h0f")
                            nc.vector.tensor_scalar_add(
                                f32t[:], ps[:], e0T_s[:, dbase + k:dbase + k + 1])
                            nc.vector.tensor_copy(hst[0][:, k * B:(k + 1) * B],
                                                  f32t[:])
                    nc.vector.memset(c_f[:], 0.0)
                    nc.vector.memset(c_b[:], 0.0)

                if phases < 2:
                    raise _Stop

                # encoder recurrence; gate cols [i(64) f(64) o(64) g(64)]
                GE = KE * 32  # 64
                with tc.tile_pool(name="eps0", bufs=2, space="PSUM") as eps0, \
                     tc.tile_pool(name="eps1", bufs=2, space="PSUM") as eps1, \
                     tc.tile_pool(name="enc_sb", bufs=3) as esb:
                    for step in range(S):
                        for d, (hst, cst, wih_s, whh_s, epsp, kk0) in enumerate([
                                (h_f, c_f, wih_f_s, whh_f_s, eps0, 0),
                                (h_b, c_b, wih_b_s, whh_b_s, eps1, KE)]):
                            s_in = step if d == 0 else S - 1 - step
                            hcur = hst[step % 2]
                            hnxt = hst[(step + 1) % 2]
                            emb = encT[:, 0, s_in * B:(s_in + 1) * B]
                            ps = epsp.tile([128, ME * 32], F32, tag=f"g{d}",
                                           name=f"g{d}")
                            for mi in range(ME):
                                oc = ps[:, mi * 32:(mi + 1) * 32]
                                nc.tensor.matmul(
                                    oc, wih_s[:, mi * 128:(mi + 1) * 128], emb,
                                    start=True, stop=False)
                                for k in range(KE):
                                    nc.tensor.matmul(
                                        oc,
                                        whh_s[:, (mi * KE + k) * 128:
                                              (mi * KE + k + 1) * 128],
                                        hcur[:, k * B:(k + 1) * B],
                                        start=False,
                                        stop=(k == KE - 1 and not bias_mode))
                                if bias_mode:
                                    bw = brow_f_s if d == 0 else brow_b_s
                                    nc.tensor.matmul(
                                        oc, bw[:, mi * 128:(mi + 1) * 128],
                                        ones_s[:, :], start=False, stop=True)
                            sg = esb.tile([128, 3 * GE], BF16, tag=f"sg{d}",
                                          name=f"sg{d}")
                            nc.scalar.activation(sg[:], ps[:, 0:3 * GE], AF.Sigmoid)
                            gt = esb.tile([128, GE], BF16, tag=f"gt{d}",
                                          name=f"gt{d}")
                            nc.scalar.activation(gt[:], ps[:, 3 * GE:4 * GE],
                                                 AF.Tanh)
                            t1 = esb.tile([128, GE], F32, tag=f"t1{d}",
                                          name=f"t1{d}")
                            nc.vector.tensor_tensor(t1[:], sg[:, GE:2 * GE],
                                                    cst[:], ALU.mult)
                            t2 = esb.tile([128, GE], F32, tag=f"t2{d}",
                                          name=f"t2{d}")
                            nc.vector.tensor_tensor(t2[:], sg[:, 0:GE], gt[:],
                                                    ALU.mult)
                            nc.vector.tensor_tensor(cst[:], t1[:], t2[:], ALU.add)
                            tct = esb.tile([128, GE], BF16, tag=f"tc{d}",
                                           name=f"tc{d}")
                            nc.scalar.activation(tct[:], cst[:], AF.Tanh)
                            nc.vector.tensor_tensor(hnxt[:], sg[:, 2 * GE:3 * GE],
                                                    tct[:], ALU.mult)
                            s_mem = s_in
                            nc.vector.tensor_copy(
                                memT4[:, kk0:kk0 + KE, :, s_mem],
                                hnxt[:].rearrange("p (k b) -> p k b", k=KE))

            if phases < 3:
                raise _Stop
            # ============ c_t / h_t decoder init ============
            h_d = [spool.tile([128, KD * B], BF16, tag=f"h_d{j}", name=f"h_d{j}")
                   for j in range(2)]
            c_d = spool.tile([128, KD * B], F32, tag="c_d", name="c_d")
            ccT = spool.tile([128, KD * B], BF16, tag="ccT", name="ccT")
            nc.vector.tensor_copy(ccT[:, 0:KE * B], c_f[:])
            nc.vector.tensor_copy(ccT[:, KE * B:2 * KE * B], c_b[:])
            with tc.tile_pool(name="ct_ps", bufs=2, space="PSUM") as cps, \
                 tc.tile_pool(name="ct_sb", bufs=2) as csb:
                for m in range(KD):
                    ps = cps.tile([128, B], F32, tag="ct", name="ct")
                    for k in range(KD):
                        lt = wtr_s[:, (k * KD + m) * 128:(k * KD + m + 1) * 128]
                        nc.tensor.matmul(ps[:], lt, ccT[:, k * B:(k + 1) * B],
                                         start=(k == 0), stop=(k == KD - 1))
                    nc.scalar.activation(c_d[:, m * B:(m + 1) * B], ps[:],
                                         AF.Prelu, alpha=0.1)
                for k in range(KD):
                    ps = cps.tile([128, B], F32, tag="ct", name="ct")
                    nc.tensor.matmul(ps[:], diff_s_s[:, k * 128:(k + 1) * 128],
                                     lab_d_s[:, :], start=True, stop=True)
                    f32t = csb.tile([128, B], F32, tag="h0d", name="h0d")
                    nc.vector.tensor_scalar_add(f32t[:], ps[:], s0T_s[:, k:k + 1])
                    nc.vector.tensor_copy(h_d[0][:, k * B:(k + 1) * B], f32t[:])

            if phases < 4:
                raise _Stop
            # ============ decoder + interleaved post-work ============
            GD = KD * 32  # 128
            with tc.tile_pool(name="dps", bufs=2, space="PSUM") as dps, \
                 tc.tile_pool(name="aps", bufs=2, space="PSUM") as apsp, \
                 tc.tile_pool(name="tps", bufs=2, space="PSUM") as tpsp, \
                 tc.tile_pool(name="sps", bufs=2, space="PSUM") as spsp, \
                 tc.tile_pool(name="dec_sb", bufs=3) as dsb, \
                 tc.tile_pool(name="at_sb", bufs=3) as asb, \
                 tc.tile_pool(name="hblk", bufs=2) as hbp, \
                 tc.tile_pool(name="cblk", bufs=1) as cbp, \
                 tc.tile_pool(name="mblk", bufs=1) as mbp, \
                 tc.tile_pool(name="lgp", bufs=2) as lgp:

                hblk_t = {}
                cblk_t = {}
                mblk_t = {}

                def unit_memN(b):
                    def emit():
                        for k in range(2 * KE):
                            mn_ps = tpsp.tile([128, 128], BF16, tag="tp",
                                              name="mnp", padded_shape=[128, 128])
                            nc.tensor.transpose(mn_ps[:], memT4[:, k, b, :],
                                                ident[:])
                            nc.vector.tensor_copy(
                                memN[:, (b * 2 * KE + k) * 128:
                                     (b * 2 * KE + k + 1) * 128], mn_ps[:])
                    return emit

                def unit_score(tau, bg):
                    def emit():
                        Hb = hblk_t[tau]
                        H4 = Hb[:].rearrange("p (k b t) -> p k b t", k=KD, b=B)
                        b0 = bg * BG
                        a_ps = apsp.tile([128, S], F32, tag="a", name="a")
                        for j in range(BG):
                            for k in range(KD):
                                nc.tensor.matmul(
                                    a_ps[32 * j:32 * (j + 1), :],
                                    H4[:, k, b0 + j, :], memT4[:, k, b0 + j, :],
                                    start=(k == 0), stop=(k == KD - 1),
                                    tile_position=(0, 32 * j))
                        mx = asb.tile([128, 1], F32, tag="mx", name="mx")
                        nc.vector.tensor_reduce(mx[:], a_ps[:], AX.X, ALU.max,
                                                negate=True)
                        mx2 = asb.tile([128, 1], F32, tag="mx2", name="mx2")
                        nc.scalar.mul(mx2[:], mx[:], SCALE)
                        ex = asb.tile([128, S], F32, tag="ex", name="ex")
                        den = asb.tile([128, 1], F32, tag="den", name="den")
                        nc.scalar.activation(ex[:], a_ps[:], AF.Exp, bias=mx2[:],
                                             scale=SCALE, accum_out=den[:])
                        rec = asb.tile([128, 1], F32, tag="rec", name="rec")
                        nc.vector.reciprocal(rec[:], den[:])
                        p_sb = asb.tile([128, S], BF16, tag="p", name="p")
                        nc.vector.tensor_scalar_mul(p_sb[:], ex[:], rec[:])
                        pt_ps = tpsp.tile([128, 128], BF16, tag="tp", name="pt",
                                          padded_shape=[128, 128])
                        nc.tensor.transpose(pt_ps[:], p_sb[:], ident[:])
                        pT = asb.tile([128, 128], BF16, tag="pT", name="pT")
                        nc.vector.tensor_copy(pT[:], pt_ps[:])
                        # ctx for the 4 b's of this group
                        Cb = cblk_t[tau]
                        for j in range(BG):
                            b = b0 + j
                            for k in range(KD):
                                c_ps = spsp.tile([128, 512], F32, tag="sp",
                                                 name="cx")
                                nc.tensor.matmul(
                                    c_ps[:, 0:Tc],
                                    memN[:, (b * 2 * KE + k) * 128:
                                         (b * 2 * KE + k + 1) * 128],
                                    pT[:, 32 * j:32 * (j + 1)],
                                    start=True, stop=True)
                                nc.vector.tensor_copy(
                                    Cb[:, k, b * Tc:(b + 1) * Tc],
                                    c_ps[:, 0:Tc])
                    return emit

                def unit_mid(tau, half, m):
                    def emit():
                        Hb = hblk_t[tau]
                        Hf = Hb[:].rearrange("p (k bt) -> p k bt", k=KD)
                        Cb = cblk_t[tau]
                        Mb = mblk_t[tau]
                        ps = spsp.tile([128, 512], F32, tag="sp", name="md")
                        c0 = half * 512
                        for k in range(KD):
                            lt = wf1_s[:, (k * KD + m) * 128:
                                       (k * KD + m + 1) * 128]
                            nc.tensor.matmul(ps[:], lt, Hf[:, k, c0:c0 + 512],
                                             start=(k == 0), stop=False)
                        for k in range(KD):
                            kk = KD + k
                            lt = wf1_s[:, (kk * KD + m) * 128:
                                       (kk * KD + m + 1) * 128]
                            nc.tensor.matmul(ps[:], lt, Cb[:, k, c0:c0 + 512],
                                             start=False, stop=(k == KD - 1))
                        nc.scalar.activation(Mb[:, m, c0:c0 + 512], ps[:],
                                             AF.Prelu, alpha=0.1,
                                             bias=b1c_s[:, m:m + 1])
                    return emit

                def unit_vocab(tau, btm):
                    def emit():
                        Mb = mblk_t[tau]
                        wf2v = wf2_s[:].rearrange("p (k v) -> p k v", k=KD)
                        b0 = btm * BG
                        t0 = tau * Tc
                        HV = VS // 2
                        for hf in range(2):
                            lgrow = lgp.tile([128, HV], BF16, tag="lgr",
                                             name="lgr")
                            for cch in range(HV // 500):
                                w0 = hf * HV + cch * 500
                                lg = spsp.tile([128, 512], F32, tag="sp",
                                               name="lg")
                                for k in range(KD):
                                    nc.tensor.matmul(
                                        lg[:, 0:500],
                                        Mb[:, k, btm * 128:(btm + 1) * 128],
                                        wf2v[:, k, w0:w0 + 500],
                                        start=(k == 0), stop=(k == KD - 1))
                                nc.vector.tensor_copy(
                                    lgrow[:, cch * 500:(cch + 1) * 500],
                                    lg[:, 0:500])
                            for j in range(BG):
                                nc.sync.dma_start(
                                    out.ap()[b0 + j, t0:t0 + Tc,
                                             hf * HV:(hf + 1) * HV],
                                    lgrow[Tc * j:Tc * (j + 1), :])
                    return emit

                def block_units(tau):
                    u = []
                    for bg in range(B // BG):
                        u.append(unit_score(tau, bg))
                    for half in range(2):
                        for m in range(KD):
                            u.append(unit_mid(tau, half, m))
                    for btm in range(B * Tc // 128):
                        u.append(unit_vocab(tau, btm))
                    return u

                queue = [unit_memN(b) for b in range(B)] if phases >= 5 else []

                for t in range(T):
                    tau = t // Tc
                    trel = t % Tc
                    if trel == 0:
                        hblk_t[tau] = hbp.tile([128, KD * B * Tc], BF16,
                                               tag="hb", name=f"hb{tau}")
                        if phases >= 5:
                            cblk_t[tau] = cbp.tile([128, KD, B * Tc], BF16,
                                                   tag="cb", name=f"cb{tau}")
                        if phases >= 6:
                            mblk_t[tau] = mbp.tile([128, KD, B * Tc], BF16,
                                                   tag="mb", name=f"mb{tau}")
                    hcur = h_d[t % 2]
                    hnxt = h_d[(t + 1) % 2]
                    emb = decT[:, 0, t * B:(t + 1) * B]
                    ps = dps.tile([128, MD * 32], F32, tag="gd", name="gd")
                    for mi in range(MD):
                        oc = ps[:, mi * 32:(mi + 1) * 32]
                        nc.tensor.matmul(oc, wih_d_s[:, mi * 128:(mi + 1) * 128],
                                         emb, start=True, stop=False)
                        for k in range(KD):
                            nc.tensor.matmul(
                                oc,
                                whh_d_s[:, (mi * KD + k) * 128:
                                        (mi * KD + k + 1) * 128],
                                hcur[:, k * B:(k + 1) * B],
                                start=False,
                                stop=(k == KD - 1 and not bias_mode))
                        if bias_mode:
                            nc.tensor.matmul(oc,
                                             brow_d_s[:, mi * 128:(mi + 1) * 128],
                                             ones_s[:, :], start=False, stop=True)
                    sg = dsb.tile([128, 3 * GD], BF16, tag="sgd", name="sgd")
                    nc.scalar.activation(sg[:], ps[:, 0:3 * GD], AF.Sigmoid)
                    gt = dsb.tile([128, GD], BF16, tag="gtd", name="gtd")
                    nc.scalar.activation(gt[:], ps[:, 3 * GD:4 * GD], AF.Tanh)
                    t1 = dsb.tile([128, GD], F32, tag="t1d", name="t1d")
                    nc.vector.tensor_tensor(t1[:], sg[:, GD:2 * GD], c_d[:],
                                            ALU.mult)
                    t2 = dsb.tile([128, GD], F32, tag="t2d", name="t2d")
                    nc.vector.tensor_tensor(t2[:], sg[:, 0:GD], gt[:], ALU.mult)
                    nc.vector.tensor_tensor(c_d[:], t1[:], t2[:], ALU.add)
                    tct = dsb.tile([128, GD], BF16, tag="tcd", name="tcd")
                    nc.scalar.activation(tct[:], c_d[:], AF.Tanh)
                    nc.vector.tensor_tensor(hnxt[:], sg[:, 2 * GD:3 * GD],
                                            tct[:], ALU.mult)
                    Hb4 = hblk_t[tau][:].rearrange("p (k b t) -> p k b t",
                                                   k=KD, b=B)
                    nc.vector.tensor_copy(
                        Hb4[:, :, :, trel],
                        hnxt[:].rearrange("p (k b) -> p k b", k=KD))
                    # interleave one unit of the previous block's post-work
                    if queue:
                        queue.pop(0)()
                    if trel == Tc - 1 and phases >= 5:
                        queue.extend(block_units(tau)
                                     if phases >= 6 else
                                     [unit_score(tau, bg)
                                      for bg in range(B // BG)])
                # tail: flush remaining units (last block)
                for u in queue:
                    u()
        except _Stop:
            pass
    nc.compile()
    return nc


def prep_inputs(i, S=128, T=128, V=32000, VS=4000, fp8=False):
    def as_np(x, dt=np.float32):
        return np.ascontiguousarray(np.asarray(x), dtype=dt)

    whh_np = f8e4 if fp8 else bf16
    tok = as_np(i["tok_emb"]).astype(bf16)

    def idx_prep(flat):
        a = flat.astype(np.int16).reshape(-1, 16).T
        return np.ascontiguousarray(np.tile(a, (8, 1)))

    inp = as_np(i["inp"], np.int64)
    x = as_np(i["x"], np.int64)
    idx_e = idx_prep(inp.T.reshape(-1))
    dmat = np.zeros((B, T), np.int64)
    dmat[:, 1:] = x[:, :T - 1]
    idx_d = idx_prep(dmat.T.reshape(-1))

    startT = as_np(i["start_emb"]).reshape(D_EMB, 1).astype(bf16)
    est = as_np(i["enc_style_emb"])
    diff_e = (est[1] - est[0]).reshape(1, -1).astype(bf16)
    e0T = np.ascontiguousarray(est[0].reshape(KD, 128).T)
    sty = as_np(i["style_emb"])
    diff_s = (sty[1] - sty[0]).reshape(1, -1).astype(bf16)
    s0T = np.ascontiguousarray(sty[0].reshape(KD, 128).T)
    lab_i = as_np(i["label_i"], np.float32).reshape(1, B).astype(bf16)
    lab_d = as_np(i["label"], np.float32).reshape(1, B).astype(bf16)

    def wihP(w, nm, perm):
        # w [4H, 128] -> [128, nm*128], tile mi = chunk perm[mi], lhsT layout
        a = w.reshape(nm, 128, 128)          # [m, out, in]
        a = a[perm]                          # permuted
        return np.ascontiguousarray(a.transpose(2, 0, 1).reshape(128, nm * 128)
                                    ).astype(bf16)

    def whhP(w, nk, nm, perm):
        # w [4H, H] -> [128, nm*nk*128], (m-perm, k)-major
        a = w.reshape(nm, 128, nk, 128)      # [m, out, k, in]
        a = a[perm]
        a = a.transpose(3, 0, 2, 1)          # [in, m, k, out]
        return np.ascontiguousarray(a.reshape(128, nm * nk * 128)).astype(whh_np)

    def whhT(w, nk, nm):
        # (k, m)-major, unpermuted (for wtr / wf1)
        a = w.reshape(nm, 128, nk, 128)
        a = a.transpose(3, 2, 0, 1)
        return np.ascontiguousarray(a.reshape(128, nk * nm * 128)).astype(bf16)

    wih_f = wihP(as_np(i["Wih_f"]), ME, PERM_E)
    wih_b = wihP(as_np(i["Wih_b"]), ME, PERM_E)
    wih_d = wihP(as_np(i["Wih_d"]), MD, PERM_D)
    whh_f = whhP(as_np(i["Whh_f"]), KE, ME, PERM_E)
    whh_b = whhP(as_np(i["Whh_b"]), KE, ME, PERM_E)
    whh_d = whhP(as_np(i["Whh_d"]), KD, MD, PERM_D)
    wtr = whhT(as_np(i["W_tr"]), KD, KD)
    wf1 = whhT(as_np(i["W_f1"]), 8, KD)
    wf2_full = as_np(i["W_f2"])
    b1c = np.ascontiguousarray(as_np(i["b_f1"]).reshape(KD, 128).T)

    bs_f = as_np(i["bih_f"]) + as_np(i["bhh_f"])
    bs_b = as_np(i["bih_b"]) + as_np(i["bhh_b"])
    bs_d = as_np(i["bih_d"]) + as_np(i["bhh_d"])
    bias_mode = bool(np.any(bs_f) or np.any(bs_b) or np.any(bs_d))

    common = dict(tokb=tok, idx_e=idx_e, idx_d=idx_d, startT=startT,
                  diff_e=diff_e, e0T=e0T, lab_i=lab_i,
                  diff_s=diff_s, s0T=s0T, lab_d=lab_d,
                  wih_f=wih_f, wih_b=wih_b, wih_d=wih_d,
                  whh_f=whh_f, whh_b=whh_b, whh_d=whh_d,
                  wtr=wtr, wf1=wf1, b1c=b1c)
    if bias_mode:
        def brow(v, nm, perm):
            a = v.reshape(nm, 128)[perm]
            return np.ascontiguousarray(a.reshape(1, nm * 128)).astype(bf16)
        common.update(brow_f=brow(bs_f, ME, PERM_E), brow_b=brow(bs_b, ME, PERM_E),
                      brow_d=brow(bs_d, MD, PERM_D))

    in_maps = []
    for c in range(N_CORES):
        shard = wf2_full[c * VS:(c + 1) * VS]
        a = shard.reshape(VS, KD, 128)
        wf2 = np.ascontiguousarray(a.transpose(2, 1, 0).reshape(128, KD * VS)
                                   ).astype(bf16)
        in_maps.append(dict(common, wf2=wf2))
    return in_maps, bias_mode


_NC_CACHE = {}
_FP8 = False


def kernel(**inputs):
    in_maps, bias_mode = prep_inputs(inputs, fp8=_FP8)
    key = (bias_mode, _FP8)
    if key not in _NC_CACHE:
        _NC_CACHE[key] = build(fp8=_FP8, bias_mode=bias_mode)
    nc = _NC_CACHE[key]
    res = run_bass_kernel_spmd(nc, in_maps, core_ids=list(range(N_CORES)))
    return np.concatenate([r["out"].astype(np.float32) for r in res.results],
                          axis=2)


# revision 19
# speedup vs baseline: 1.5487x; 1.1617x over previous
"""DenoiseLSTM Trainium2 kernel (8 NeuronCores, SPMD) — v2.

Strategy vs v1 baseline:
- Input projections are folded into the recurrent loops as extra matmul
  accumulation steps (embeddings live in SBUF from the gather), killing the
  DMA-scatter projection phase and the per-step g=ps+xt vector add.
- Gate m-chunks are permuted host-side to [i, f, o, g] so one sigmoid
  activation covers i/f/o straight out of PSUM; tanh covers g.
- Attention + FFN + vocab projection are restructured per 32-step t-block
  and emitted interleaved with decoder steps, so the Tile scheduler fills
  the decoder's dependency-stall gaps with post-work.
- Attention scores pack 4 batch rows into one [128,S] PSUM tile via
  tile_position column packing; softmax runs on full 128 partitions.
- Output is written bf16 and upcast on host (halves output DMA).
- Optional: recurrent Whh weights in fp8-e4m3 (halves LDWEIGHTS traffic);
  CPU-simulated rel err 0.0079 vs 2e-2 budget.

Replicated recurrences on all 8 cores; vocab projection sharded over V
(4000 columns per core); host concatenates V-shards.
"""
import sys

sys.path.insert(0, "/opt/trn_rl_repo")

from contextlib import ExitStack

import numpy as np
import ml_dtypes

import concourse.bass as bass
import concourse.bacc as bacc
import concourse.mybir as mybir
import concourse.tile as tile
from concourse.bass_utils import run_bass_kernel_spmd
from concourse.masks import make_identity

bf16 = ml_dtypes.bfloat16
f8e4 = ml_dtypes.float8_e4m3
F32 = mybir.dt.float32
BF16 = mybir.dt.bfloat16
FP8E4 = mybir.dt.float8e4
I16 = mybir.dt.int16
AF = mybir.ActivationFunctionType
ALU = mybir.AluOpType
AX = mybir.AxisListType

B = 32
D_EMB = 128
D_ENC = 256
D_DEC = 512
N_CORES = 8
KE = D_ENC // 128   # 2
KD = D_DEC // 128   # 4
ME = 4 * D_ENC // 128  # 8 gate chunks (encoder)
MD = 4 * D_DEC // 128  # 16 gate chunks (decoder)
# m-chunk permutation: [i-chunks, f-chunks, o-chunks, g-chunks]
PERM_E = [0, 1, 2, 3, 6, 7, 4, 5]
PERM_D = [0, 1, 2, 3, 4, 5, 6, 7, 12, 13, 14, 15, 8, 9, 10, 11]


class _Stop(Exception):
    pass


def build(S=128, T=128, V=32000, VS=4000, phases=6, fp8=False, bias_mode=False):
    NI_E = B * S
    NI_D = B * T
    Tc = 32                      # timesteps per post-work block
    NBLK = T // Tc               # 4
    BG = 4                       # batch rows packed per score tile
    SCALE = 1.0 / float(np.sqrt(np.float32(2 * D_ENC)))
    WHH_DT = FP8E4 if fp8 else BF16

    nc = bacc.Bacc("TRN2", target_bir_lowering=False, debug=False)

    # ---- external inputs ----
    tokb = nc.dram_tensor("tokb", [V, D_EMB], BF16, kind="ExternalInput")
    idx_e = nc.dram_tensor("idx_e", [128, NI_E // 16], I16, kind="ExternalInput")
    idx_d = nc.dram_tensor("idx_d", [128, NI_D // 16], I16, kind="ExternalInput")
    startT = nc.dram_tensor("startT", [128, 1], BF16, kind="ExternalInput")
    diff_e = nc.dram_tensor("diff_e", [1, 2 * D_ENC], BF16, kind="ExternalInput")
    e0T = nc.dram_tensor("e0T", [128, KD], F32, kind="ExternalInput")
    lab_i = nc.dram_tensor("lab_i", [1, B], BF16, kind="ExternalInput")
    diff_s = nc.dram_tensor("diff_s", [1, D_DEC], BF16, kind="ExternalInput")
    s0T = nc.dram_tensor("s0T", [128, KD], F32, kind="ExternalInput")
    lab_d = nc.dram_tensor("lab_d", [1, B], BF16, kind="ExternalInput")
    # input-projection weights (bf16), m-perm-major
    wih_f = nc.dram_tensor("wih_f", [128, ME * 128], BF16, kind="ExternalInput")
    wih_b = nc.dram_tensor("wih_b", [128, ME * 128], BF16, kind="ExternalInput")
    wih_d = nc.dram_tensor("wih_d", [128, MD * 128], BF16, kind="ExternalInput")
    # recurrent weights, (m-perm, k)-major, optionally fp8
    whh_f = nc.dram_tensor("whh_f", [128, ME * KE * 128], WHH_DT, kind="ExternalInput")
    whh_b = nc.dram_tensor("whh_b", [128, ME * KE * 128], WHH_DT, kind="ExternalInput")
    whh_d = nc.dram_tensor("whh_d", [128, MD * KD * 128], WHH_DT, kind="ExternalInput")
    wtr = nc.dram_tensor("wtr", [128, KD * KD * 128], BF16, kind="ExternalInput")
    wf1 = nc.dram_tensor("wf1", [128, 8 * KD * 128], BF16, kind="ExternalInput")
    wf2 = nc.dram_tensor("wf2", [128, KD * VS], BF16, kind="ExternalInput")
    b1a = nc.dram_tensor("b1a", [128, KD], F32, kind="ExternalInput")
    b1h = nc.dram_tensor("b1h", [128, KD], F32, kind="ExternalInput")
    if bias_mode:
        brow_f = nc.dram_tensor("brow_f", [1, ME * 128], BF16, kind="ExternalInput")
        brow_b = nc.dram_tensor("brow_b", [1, ME * 128], BF16, kind="ExternalInput")
        brow_d = nc.dram_tensor("brow_d", [1, MD * 128], BF16, kind="ExternalInput")

    out = nc.dram_tensor("out", [B, T, VS], BF16, kind="ExternalOutput")

    with tile.TileContext(nc) as tc, ExitStack() as ctx:
        wpool = ctx.enter_context(tc.tile_pool(name="weights", bufs=1))
        spool = ctx.enter_context(tc.tile_pool(name="state", bufs=1))
        big = ctx.enter_context(tc.tile_pool(name="big", bufs=1))

        def load(dram, shape, dtype, tag):
            t = wpool.tile(shape, dtype, tag=tag, name=tag)
            nc.sync.dma_start(t[:], dram[:, :])
            return t

        # gathers first: idx DMAs + encoder-token gather start immediately,
        # overlapping the weight streams
        idx_e_s = wpool.tile([128, NI_E // 16], I16, tag="idx_e", name="idx_e")
        nc.sync.dma_start(idx_e_s[:], idx_e[:, :])
        idx_d_s = wpool.tile([128, NI_D // 16], I16, tag="idx_d", name="idx_d")
        nc.sync.dma_start(idx_d_s[:], idx_d[:, :])
        encT = big.tile([128, 1, NI_E], BF16, tag="encT", name="encT")
        nc.gpsimd.dma_gather(encT[:], tokb[:, :], idx_e_s[:], NI_E, NI_E,
                             D_EMB, transpose=True, single_packet=False)
        decT = big.tile([128, 1, NI_D], BF16, tag="decT", name="decT")
        nc.gpsimd.dma_gather(decT[:], tokb[:, :], idx_d_s[:], NI_D, NI_D,
                             D_EMB, transpose=True, single_packet=False)

        wih_f_s = load(wih_f, [128, ME * 128], BF16, "wih_f")
        wih_b_s = load(wih_b, [128, ME * 128], BF16, "wih_b")
        wih_d_s = load(wih_d, [128, MD * 128], BF16, "wih_d")
        whh_f_s = load(whh_f, [128, ME * KE * 128], WHH_DT, "whh_f")
        whh_b_s = load(whh_b, [128, ME * KE * 128], WHH_DT, "whh_b")
        whh_d_s = load(whh_d, [128, MD * KD * 128], WHH_DT, "whh_d")
        wtr_s = load(wtr, [128, KD * KD * 128], BF16, "wtr")
        wf1_s = load(wf1, [128, 8 * KD * 128], BF16, "wf1")
        wf2_s = load(wf2, [128, KD * VS], BF16, "wf2")
        b1a_s = load(b1a, [128, KD], F32, "b1a")
        b1h_s = load(b1h, [128, KD], F32, "b1h")
        e0T_s = load(e0T, [128, KD], F32, "e0T")
        s0T_s = load(s0T, [128, KD], F32, "s0T")
        startT_s = load(startT, [128, 1], BF16, "startT")
        ident = wpool.tile([128, 128], BF16, tag="ident", name="ident")
        make_identity(nc, ident)

        diff_e_s = wpool.tile([1, 2 * D_ENC], BF16, tag="diff_e", name="diff_e")
        nc.sync.dma_start(diff_e_s[:], diff_e[:, :])
        diff_s_s = wpool.tile([1, D_DEC], BF16, tag="diff_s", name="diff_s")
        nc.sync.dma_start(diff_s_s[:], diff_s[:, :])
        lab_i_s = wpool.tile([1, B], BF16, tag="lab_i", name="lab_i")
        nc.sync.dma_start(lab_i_s[:], lab_i[:, :])
        lab_d_s = wpool.tile([1, B], BF16, tag="lab_d", name="lab_d")
        nc.sync.dma_start(lab_d_s[:], lab_d[:, :])
        if bias_mode:
            brow_f_s = load(brow_f, [1, ME * 128], BF16, "brow_f")
            brow_b_s = load(brow_b, [1, ME * 128], BF16, "brow_b")
            brow_d_s = load(brow_d, [1, MD * 128], BF16, "brow_d")
            ones_s = wpool.tile([1, B], BF16, tag="ones", name="ones")
            nc.vector.memset(ones_s[:], 1.0)

        # decoder step-0 input = start_emb (broadcast over b)
        nc.vector.tensor_copy(decT[:, 0, 0:B],
                              startT_s[:, 0:1].to_broadcast((128, B)))

        # ---------- shared state ----------
        mem_T = big.tile([128, 2 * KE * B * S], BF16, tag="mem_T", name="mem_T")
        memT4 = mem_T[:].rearrange("p (k b s) -> p k b s", k=2 * KE, b=B)
        memN = big.tile([128, B * 2 * KE * 128], BF16, tag="memN", name="memN")

        h_f = [spool.tile([128, KE * B], BF16, tag=f"h_f{j}", name=f"h_f{j}")
               for j in range(2)]
        h_b = [spool.tile([128, KE * B], BF16, tag=f"h_b{j}", name=f"h_b{j}")
               for j in range(2)]
        c_f = spool.tile([128, KE * B], F32, tag="c_f", name="c_f")
        c_b = spool.tile([128, KE * B], F32, tag="c_b", name="c_b")

        try:
            # ============ encoder ============
            # h0 init via outer product + broadcast add
            with tc.tile_pool(name="init_ps", bufs=2, space="PSUM") as ips, \
                 tc.tile_pool(name="init_sb", bufs=2) as isb:
                for d, (hst, dbase) in enumerate([(h_f, 0), (h_b, KE)]):
                    for k in range(KE):
                        ps = ips.tile([128, B], F32, tag="i", name="i")
                        col = (dbase + k) * 128
                        nc.tensor.matmul(ps[:], diff_e_s[:, col:col + 128],
                                         lab_i_s[:, :], start=True, stop=True)
                        f32t = isb.tile([128, B], F32, tag="h0f", name="h0f")
                        nc.vector.tensor_scalar_add(
                            f32t[:], ps[:], e0T_s[:, dbase + k:dbase + k + 1])
                        nc.vector.tensor_copy(hst[0][:, k * B:(k + 1) * B],
                                              f32t[:])
                nc.vector.memset(c_f[:], 0.0)
                nc.vector.memset(c_b[:], 0.0)

            if phases < 2:
                raise _Stop

            # encoder recurrence; bank A = [i f], bank B = [o g]
            GE = KE * 32  # 64
            MH = ME // 2
            with tc.tile_pool(name="eps0a", bufs=2, space="PSUM") as eps0a, \
                 tc.tile_pool(name="eps0b", bufs=2, space="PSUM") as eps0b, \
                 tc.tile_pool(name="eps1a", bufs=2, space="PSUM") as eps1a, \
                 tc.tile_pool(name="eps1b", bufs=2, space="PSUM") as eps1b, \
                 tc.tile_pool(name="enc_sb", bufs=3) as esb:
                epools = [(eps0a, eps0b), (eps1a, eps1b)]
                for step in range(S):
                    for d, (hst, cst, wih_s, whh_s, kk0) in enumerate([
                            (h_f, c_f, wih_f_s, whh_f_s, 0),
                            (h_b, c_b, wih_b_s, whh_b_s, KE)]):
                        pa, pb = epools[d]
                        s_in = step if d == 0 else S - 1 - step
                        hcur = hst[step % 2]
                        hnxt = hst[(step + 1) % 2]
                        emb = encT[:, 0, s_in * B:(s_in + 1) * B]
                        psa = pa.tile([128, MH * 32], F32, tag=f"ga{d}",
                                      name=f"ga{d}")
                        psb = pb.tile([128, MH * 32], F32, tag=f"gb{d}",
                                      name=f"gb{d}")
                        for mi in range(ME):
                            ps = psa if mi < MH else psb
                            oc = ps[:, (mi % MH) * 32:(mi % MH + 1) * 32]
                            nc.tensor.matmul(
                                oc, wih_s[:, mi * 128:(mi + 1) * 128], emb,
                                start=True, stop=False)
                            for k in range(KE):
                                nc.tensor.matmul(
                                    oc,
                                    whh_s[:, (mi * KE + k) * 128:
                                          (mi * KE + k + 1) * 128],
                                    hcur[:, k * B:(k + 1) * B],
                                    start=False,
                                    stop=(k == KE - 1 and not bias_mode))
                            if bias_mode:
                                bw = brow_f_s if d == 0 else brow_b_s
                                nc.tensor.matmul(
                                    oc, bw[:, mi * 128:(mi + 1) * 128],
                                    ones_s[:, :], start=False, stop=True)
                        # bank A: [i(GE) f(GE)]; bank B: [o(GE) g(GE)]
                        sgif = esb.tile([128, 2 * GE], F32, tag=f"sg{d}",
                                        name=f"sg{d}")
                        nc.scalar.activation(sgif[:], psa[:], AF.Sigmoid)
                        gt = esb.tile([128, GE], F32, tag=f"gt{d}",
                                      name=f"gt{d}")
                        nc.scalar.activation(gt[:], psb[:, GE:2 * GE], AF.Tanh)
                        t1 = esb.tile([128, GE], F32, tag=f"t1{d}",
                                      name=f"t1{d}")
                        nc.vector.tensor_tensor(t1[:], sgif[:, GE:2 * GE],
                                                cst[:], ALU.mult)
                        t2 = esb.tile([128, GE], F32, tag=f"t2{d}",
                                      name=f"t2{d}")
                        nc.vector.tensor_tensor(t2[:], sgif[:, 0:GE], gt[:],
                                                ALU.mult)
                        nc.vector.tensor_tensor(cst[:], t1[:], t2[:], ALU.add)
                        sgo = esb.tile([128, GE], F32, tag=f"so{d}",
                                       name=f"so{d}")
                        nc.scalar.activation(sgo[:], psb[:, 0:GE], AF.Sigmoid)
                        tct = esb.tile([128, GE], F32, tag=f"tc{d}",
                                       name=f"tc{d}")
                        nc.scalar.activation(tct[:], cst[:], AF.Tanh)
                        nc.vector.tensor_tensor(hnxt[:], sgo[:], tct[:],
                                                ALU.mult)
                        s_mem = s_in
                        nc.vector.tensor_copy(
                            memT4[:, kk0:kk0 + KE, :, s_mem],
                            hnxt[:].rearrange("p (k b) -> p k b", k=KE))

            if phases < 3:
                raise _Stop
            # ============ c_t / h_t decoder init ============
            h_d = [spool.tile([128, KD * B], BF16, tag=f"h_d{j}", name=f"h_d{j}")
                   for j in range(2)]
            c_d = spool.tile([128, KD * B], F32, tag="c_d", name="c_d")
            ccT = spool.tile([128, KD * B], BF16, tag="ccT", name="ccT")
            nc.vector.tensor_copy(ccT[:, 0:KE * B], c_f[:])
            nc.vector.tensor_copy(ccT[:, KE * B:2 * KE * B], c_b[:])
            with tc.tile_pool(name="ct_ps", bufs=2, space="PSUM") as cps, \
                 tc.tile_pool(name="ct_sb", bufs=2) as csb:
                for m in range(KD):
                    ps = cps.tile([128, B], F32, tag="ct", name="ct")
                    for k in range(KD):
                        lt = wtr_s[:, (k * KD + m) * 128:(k * KD + m + 1) * 128]
                        nc.tensor.matmul(ps[:], lt, ccT[:, k * B:(k + 1) * B],
                                         start=(k == 0), stop=(k == KD - 1))
                    # lrelu(y) = 0.55 y + 0.45 |y|
                    ab = csb.tile([128, B], F32, tag="ab", name="ab")
                    nc.scalar.activation(ab[:], ps[:], AF.Abs)
                    idt = csb.tile([128, B], F32, tag="idt", name="idt")
                    nc.scalar.activation(idt[:], ps[:], AF.Identity, scale=0.55)
                    nc.vector.scalar_tensor_tensor(c_d[:, m * B:(m + 1) * B],
                                                   ab[:], 0.45, idt[:],
                                                   ALU.mult, ALU.add)
                for k in range(KD):
                    ps = cps.tile([128, B], F32, tag="ct", name="ct")
                    nc.tensor.matmul(ps[:], diff_s_s[:, k * 128:(k + 1) * 128],
                                     lab_d_s[:, :], start=True, stop=True)
                    f32t = csb.tile([128, B], F32, tag="h0d", name="h0d")
                    nc.vector.tensor_scalar_add(f32t[:], ps[:], s0T_s[:, k:k + 1])
                    nc.vector.tensor_copy(h_d[0][:, k * B:(k + 1) * B], f32t[:])

            if phases < 4:
                raise _Stop
            # ============ decoder + interleaved post-work ============
            GD = KD * 32  # 128
            with tc.tile_pool(name="dpsa", bufs=2, space="PSUM") as dpsa, \
                 tc.tile_pool(name="dpsb", bufs=2, space="PSUM") as dpsb, \
                 tc.tile_pool(name="aps", bufs=1, space="PSUM") as apsp, \
                 tc.tile_pool(name="tps", bufs=1, space="PSUM") as tpsp, \
                 tc.tile_pool(name="sps", bufs=2, space="PSUM") as spsp, \
                 tc.tile_pool(name="dec_sb", bufs=3) as dsb, \
                 tc.tile_pool(name="at_sb", bufs=3) as asb, \
                 tc.tile_pool(name="mid_sb", bufs=1) as mlsb, \
                 tc.tile_pool(name="hblk", bufs=2) as hbp, \
                 tc.tile_pool(name="cblk", bufs=1) as cbp, \
                 tc.tile_pool(name="mblk", bufs=1) as mbp, \
                 tc.tile_pool(name="lgp", bufs=2) as lgp:

                hblk_t = {}
                cblk_t = {}
                mblk_t = {}

                def unit_memN(b):
                    def emit():
                        for k in range(2 * KE):
                            mn_ps = tpsp.tile([128, 128], BF16, tag="tp",
                                              name="mnp", padded_shape=[128, 128])
                            nc.tensor.transpose(mn_ps[:], memT4[:, k, b, :],
                                                ident[:])
                            nc.vector.tensor_copy(
                                memN[:, (b * 2 * KE + k) * 128:
                                     (b * 2 * KE + k + 1) * 128], mn_ps[:])
                    return emit

                def unit_score(tau, bg):
                    def emit():
                        Hb = hblk_t[tau]
                        H4 = Hb[:].rearrange("p (k b t) -> p k b t", k=KD, b=B)
                        b0 = bg * BG
                        a_ps = apsp.tile([128, S], F32, tag="a", name="a")
                        for j in range(BG):
                            for k in range(KD):
                                nc.tensor.matmul(
                                    a_ps[32 * j:32 * (j + 1), :],
                                    H4[:, k, b0 + j, :], memT4[:, k, b0 + j, :],
                                    start=(k == 0), stop=(k == KD - 1),
                                    tile_position=(0, 32 * j))
                        mx = asb.tile([128, 1], F32, tag="mx", name="mx")
                        nc.vector.tensor_reduce(mx[:], a_ps[:], AX.X, ALU.max,
                                                negate=True)
                        mx2 = asb.tile([128, 1], F32, tag="mx2", name="mx2")
                        nc.scalar.mul(mx2[:], mx[:], SCALE)
                        ex = asb.tile([128, S], F32, tag="ex", name="ex")
                        den = asb.tile([128, 1], F32, tag="den", name="den")
                        nc.scalar.activation(ex[:], a_ps[:], AF.Exp, bias=mx2[:],
                                             scale=SCALE, accum_out=den[:])
                        rec = asb.tile([128, 1], F32, tag="rec", name="rec")
                        nc.vector.reciprocal(rec[:], den[:])
                        p_sb = asb.tile([128, S], BF16, tag="p", name="p")
                        nc.vector.tensor_scalar_mul(p_sb[:], ex[:], rec[:])
                        pt_ps = tpsp.tile([128, 128], BF16, tag="tp", name="pt",
                                          padded_shape=[128, 128])
                        nc.tensor.transpose(pt_ps[:], p_sb[:], ident[:])
                        pT = asb.tile([128, 128], BF16, tag="pT", name="pT")
                        nc.vector.tensor_copy(pT[:], pt_ps[:])
                        # ctx for the 4 b's of this group
                        Cb = cblk_t[tau]
                        for j in range(BG):
                            b = b0 + j
                            for k in range(KD):
                                c_ps = spsp.tile([128, 512], F32, tag="sp",
                                                 name="cx")
                                nc.tensor.matmul(
                                    c_ps[:, 0:Tc],
                                    memN[:, (b * 2 * KE + k) * 128:
                                         (b * 2 * KE + k + 1) * 128],
                                    pT[:, 32 * j:32 * (j + 1)],
                                    start=True, stop=True)
                                nc.vector.tensor_copy(
                                    Cb[:, k, b * Tc:(b + 1) * Tc],
                                    c_ps[:, 0:Tc])
                    return emit

                def unit_mid(tau, half, m):
                    def emit():
                        Hb = hblk_t[tau]
                        Hf = Hb[:].rearrange("p (k bt) -> p k bt", k=KD)
                        Cb = cblk_t[tau]
                        Mb = mblk_t[tau]
                        ps = spsp.tile([128, 512], F32, tag="sp", name="md")
                        c0 = half * 512
                        for k in range(KD):
                            lt = wf1_s[:, (k * KD + m) * 128:
                                       (k * KD + m + 1) * 128]
                            nc.tensor.matmul(ps[:], lt, Hf[:, k, c0:c0 + 512],
                                             start=(k == 0), stop=False)
                        for k in range(KD):
                            kk = KD + k
                            lt = wf1_s[:, (kk * KD + m) * 128:
                                       (kk * KD + m + 1) * 128]
                            nc.tensor.matmul(ps[:], lt, Cb[:, k, c0:c0 + 512],
                                             start=False, stop=(k == KD - 1))
                        ab = mlsb.tile([128, 512], F32, tag="mab", name="mab")
                        nc.scalar.activation(ab[:], ps[:], AF.Abs,
                                             bias=b1a_s[:, m:m + 1])
                        idt = mlsb.tile([128, 512], F32, tag="mid", name="mid")
                        nc.scalar.activation(idt[:], ps[:], AF.Identity,
                                             scale=0.55, bias=b1h_s[:, m:m + 1])
                        nc.vector.scalar_tensor_tensor(
                            Mb[:, m, c0:c0 + 512], ab[:], 0.45, idt[:],
                            ALU.mult, ALU.add)
                    return emit

                def unit_vocab(tau, btm):
                    def emit():
                        Mb = mblk_t[tau]
                        wf2v = wf2_s[:].rearrange("p (k v) -> p k v", k=KD)
                        b0 = btm * BG
                        t0 = tau * Tc
                        HV = VS // 2
                        for hf in range(2):
                            lgrow = lgp.tile([128, HV], BF16, tag="lgr",
                                             name="lgr")
                            for cch in range(HV // 500):
                                w0 = hf * HV + cch * 500
                                lg = spsp.tile([128, 512], F32, tag="sp",
                                               name="lg")
                                for k in range(KD):
                                    nc.tensor.matmul(
                                        lg[:, 0:500],
                                        Mb[:, k, btm * 128:(btm + 1) * 128],
                                        wf2v[:, k, w0:w0 + 500],
                                        start=(k == 0), stop=(k == KD - 1))
                                nc.vector.tensor_copy(
                                    lgrow[:, cch * 500:(cch + 1) * 500],
                                    lg[:, 0:500])
                            for j in range(BG):
                                nc.sync.dma_start(
                                    out.ap()[b0 + j, t0:t0 + Tc,
                                             hf * HV:(hf + 1) * HV],
                                    lgrow[Tc * j:Tc * (j + 1), :])
                    return emit

                MHD = MD // 2
                for t in range(T):
                    tau = t // Tc
                    trel = t % Tc
                    if trel == 0:
                        hblk_t[tau] = hbp.tile([128, KD * B * Tc], BF16,
                                               tag="hb", name=f"hb{tau}")
                    hcur = h_d[t % 2]
                    hnxt = h_d[(t + 1) % 2]
                    emb = decT[:, 0, t * B:(t + 1) * B]
                    psa = dpsa.tile([128, MHD * 32], F32, tag="gda", name="gda")
                    psb = dpsb.tile([128, MHD * 32], F32, tag="gdb", name="gdb")
                    for mi in range(MD):
                        ps = psa if mi < MHD else psb
                        oc = ps[:, (mi % MHD) * 32:(mi % MHD + 1) * 32]
                        nc.tensor.matmul(oc, wih_d_s[:, mi * 128:(mi + 1) * 128],
                                         emb, start=True, stop=False)
                        for k in range(KD):
                            nc.tensor.matmul(
                                oc,
                                whh_d_s[:, (mi * KD + k) * 128:
                                        (mi * KD + k + 1) * 128],
                                hcur[:, k * B:(k + 1) * B],
                                start=False,
                                stop=(k == KD - 1 and not bias_mode))
                        if bias_mode:
                            nc.tensor.matmul(oc,
                                             brow_d_s[:, mi * 128:(mi + 1) * 128],
                                             ones_s[:, :], start=False, stop=True)
                    # bank A: [i(GD) f(GD)]; bank B: [o(GD) g(GD)]
                    sgif = dsb.tile([128, 2 * GD], F32, tag="sgd", name="sgd")
                    nc.scalar.activation(sgif[:], psa[:], AF.Sigmoid)
                    gt = dsb.tile([128, GD], F32, tag="gtd", name="gtd")
                    nc.scalar.activation(gt[:], psb[:, GD:2 * GD], AF.Tanh)
                    t1 = dsb.tile([128, GD], F32, tag="t1d", name="t1d")
                    nc.vector.tensor_tensor(t1[:], sgif[:, GD:2 * GD], c_d[:],
                                            ALU.mult)
                    t2 = dsb.tile([128, GD], F32, tag="t2d", name="t2d")
                    nc.vector.tensor_tensor(t2[:], sgif[:, 0:GD], gt[:],
                                            ALU.mult)
                    nc.vector.tensor_tensor(c_d[:], t1[:], t2[:], ALU.add)
                    sgo = dsb.tile([128, GD], F32, tag="sod", name="sod")
                    nc.scalar.activation(sgo[:], psb[:, 0:GD], AF.Sigmoid)
                    tct = dsb.tile([128, GD], F32, tag="tcd", name="tcd")
                    nc.scalar.activation(tct[:], c_d[:], AF.Tanh)
                    nc.vector.tensor_tensor(hnxt[:], sgo[:], tct[:], ALU.mult)
                    Hb4 = hblk_t[tau][:].rearrange("p (k b t) -> p k b t",
                                                   k=KD, b=B)
                    nc.vector.tensor_copy(
                        Hb4[:, :, :, trel],
                        hnxt[:].rearrange("p (k b) -> p k b", k=KD))

                # post-work emitted AFTER the loop: lowest priority, so the
                # scheduler uses it purely as gap-filler inside the dec loop
                if phases >= 5:
                    for b in range(B):
                        unit_memN(b)()
                    for tau in range(NBLK):
                        cblk_t[tau] = cbp.tile([128, KD, B * Tc], BF16,
                                               tag="cb", name=f"cb{tau}")
                        if phases >= 6:
                            mblk_t[tau] = mbp.tile([128, KD, B * Tc], BF16,
                                                   tag="mb", name=f"mb{tau}")
                        for bg in range(B // BG):
                            unit_score(tau, bg)()
                        if phases >= 6:
                            for half in range(2):
                                for m in range(KD):
                                    unit_mid(tau, half, m)()
                            for btm in range(B * Tc // 128):
                                unit_vocab(tau, btm)()
        except _Stop:
            pass
    nc.compile()
    return nc


def prep_inputs(i, S=128, T=128, V=32000, VS=4000, fp8=False):
    def as_np(x, dt=np.float32):
        return np.ascontiguousarray(np.asarray(x), dtype=dt)

    whh_np = f8e4 if fp8 else bf16
    tok = as_np(i["tok_emb"]).astype(bf16)

    def idx_prep(flat):
        a = flat.astype(np.int16).reshape(-1, 16).T
        return np.ascontiguousarray(np.tile(a, (8, 1)))

    inp = as_np(i["inp"], np.int64)
    x = as_np(i["x"], np.int64)
    idx_e = idx_prep(inp.T.reshape(-1))
    dmat = np.zeros((B, T), np.int64)
    dmat[:, 1:] = x[:, :T - 1]
    idx_d = idx_prep(dmat.T.reshape(-1))

    startT = as_np(i["start_emb"]).reshape(D_EMB, 1).astype(bf16)
    est = as_np(i["enc_style_emb"])
    diff_e = (est[1] - est[0]).reshape(1, -1).astype(bf16)
    e0T = np.ascontiguousarray(est[0].reshape(KD, 128).T)
    sty = as_np(i["style_emb"])
    diff_s = (sty[1] - sty[0]).reshape(1, -1).astype(bf16)
    s0T = np.ascontiguousarray(sty[0].reshape(KD, 128).T)
    lab_i = as_np(i["label_i"], np.float32).reshape(1, B).astype(bf16)
    lab_d = as_np(i["label"], np.float32).reshape(1, B).astype(bf16)

    def wihP(w, nm, perm):
        # w [4H, 128] -> [128, nm*128], tile mi = chunk perm[mi], lhsT layout
        a = w.reshape(nm, 128, 128)          # [m, out, in]
        a = a[perm]                          # permuted
        return np.ascontiguousarray(a.transpose(2, 0, 1).reshape(128, nm * 128)
                                    ).astype(bf16)

    def whhP(w, nk, nm, perm):
        # w [4H, H] -> [128, nm*nk*128], (m-perm, k)-major
        a = w.reshape(nm, 128, nk, 128)      # [m, out, k, in]
        a = a[perm]
        a = a.transpose(3, 0, 2, 1)          # [in, m, k, out]
        return np.ascontiguousarray(a.reshape(128, nm * nk * 128)).astype(whh_np)

    def whhT(w, nk, nm):
        # (k, m)-major, unpermuted (for wtr / wf1)
        a = w.reshape(nm, 128, nk, 128)
        a = a.transpose(3, 2, 0, 1)
        return np.ascontiguousarray(a.reshape(128, nk * nm * 128)).astype(bf16)

    wih_f = wihP(as_np(i["Wih_f"]), ME, PERM_E)
    wih_b = wihP(as_np(i["Wih_b"]), ME, PERM_E)
    wih_d = wihP(as_np(i["Wih_d"]), MD, PERM_D)
    whh_f = whhP(as_np(i["Whh_f"]), KE, ME, PERM_E)
    whh_b = whhP(as_np(i["Whh_b"]), KE, ME, PERM_E)
    whh_d = whhP(as_np(i["Whh_d"]), KD, MD, PERM_D)
    wtr = whhT(as_np(i["W_tr"]), KD, KD)
    wf1 = whhT(as_np(i["W_f1"]), 8, KD)
    wf2_full = as_np(i["W_f2"])
    b1 = as_np(i["b_f1"])
    b1a = np.ascontiguousarray(b1.reshape(KD, 128).T)
    b1h = np.ascontiguousarray((0.55 * b1).reshape(KD, 128).T)

    bs_f = as_np(i["bih_f"]) + as_np(i["bhh_f"])
    bs_b = as_np(i["bih_b"]) + as_np(i["bhh_b"])
    bs_d = as_np(i["bih_d"]) + as_np(i["bhh_d"])
    bias_mode = bool(np.any(bs_f) or np.any(bs_b) or np.any(bs_d))

    common = dict(tokb=tok, idx_e=idx_e, idx_d=idx_d, startT=startT,
                  diff_e=diff_e, e0T=e0T, lab_i=lab_i,
                  diff_s=diff_s, s0T=s0T, lab_d=lab_d,
                  wih_f=wih_f, wih_b=wih_b, wih_d=wih_d,
                  whh_f=whh_f, whh_b=whh_b, whh_d=whh_d,
                  wtr=wtr, wf1=wf1, b1a=b1a, b1h=b1h)
    if bias_mode:
        def brow(v, nm, perm):
            a = v.reshape(nm, 128)[perm]
            return np.ascontiguousarray(a.reshape(1, nm * 128)).astype(bf16)
        common.update(brow_f=brow(bs_f, ME, PERM_E), brow_b=brow(bs_b, ME, PERM_E),
                      brow_d=brow(bs_d, MD, PERM_D))

    in_maps = []
    for c in range(N_CORES):
        shard = wf2_full[c * VS:(c + 1) * VS]
        a = shard.reshape(VS, KD, 128)
        wf2 = np.ascontiguousarray(a.transpose(2, 1, 0).reshape(128, KD * VS)
                                   ).astype(bf16)
        in_maps.append(dict(common, wf2=wf2))
    return in_maps, bias_mode


_NC_CACHE = {}
_FP8 = True


def kernel(**inputs):
    in_maps, bias_mode = prep_inputs(inputs, fp8=_FP8)
    key = (bias_mode, _FP8)
    if key not in _NC_CACHE:
        _NC_CACHE[key] = build(fp8=_FP8, bias_mode=bias_mode)
    nc = _NC_CACHE[key]
    res = run_bass_kernel_spmd(nc, in_maps, core_ids=list(range(N_CORES)))
    return np.concatenate([r["out"].astype(np.float32) for r in res.results],
                          axis=2)
